# revision 10
# baseline (speedup 1.0000x reference)
"""Trainium2 Bass kernel for nn_DecoderLayer_72327249264859.

Decoder layer: self-attn + (cross-attn || graph-attn) + FFN, each with
residual + layernorm. B=4, T=S=1024, D=1024, 16 heads, ffn=4096.

Sharding: pure data-parallel over query tokens. Core c handles batch
element b = c // 2, query rows (c % 2)*512 .. +512. Each core computes
full-length K/V for its batch element (2x duplicated K/V work, but zero
cross-core communication). The host permutes the self-attention key axis
so each core's own query tokens are the first 512 columns of x_t
(attention is invariant to key order as long as the mask rows are
permuted identically), so the same SPMD program works on every core.

Layout strategy: activations are kept *feature-major* in SBUF
([feature-dim on partitions, tokens on free]):
  - feature-major projections: lhsT = W column-panel [Din, 128] (natural),
    rhs = X_t [d, t] -> out [n, t]; no transposes anywhere;
  - token-major outputs (V for probs@V) use lhsT = X_t chunk (activations
    stationary) and rhs = W natural [Din, Dout];
  - scores are computed transposed ([s part, t free]): lhsT = K_t slice,
    rhs = Q_t slice (contraction over dh=64);
  - the additive mask (host-pre-transposed) is accumulated into the scores
    PSUM with an identity-matrix matmul;
  - softmax-Z (partition reduction) is an all-ones matmul that lands
    replicated across partitions in PSUM; 1/Z via fast DVE reciprocal;
  - layernorm mean / mean-of-squares via (1/D)-ones matmuls, same trick.
All matmuls bf16 with fp32 PSUM accumulation; heads processed in pairs
sharing PSUM tiles (partition halves).
"""

import sys

if "/opt/trn_rl_repo" not in sys.path:
    sys.path.insert(0, "/opt/trn_rl_repo")

import numpy as np
import ml_dtypes
from contextlib import ExitStack

import concourse.bacc as bacc
import concourse.mybir as mybir
from concourse.tile import TileContext

BF16 = mybir.dt.bfloat16
F32 = mybir.dt.float32
AF = mybir.ActivationFunctionType
ALU = mybir.AluOpType

B, T, S, D = 4, 1024, 1024, 1024
NH, DH = 16, 64
F = 4 * D
SCALE = DH**-0.5
EPS = 1e-5
P = 128
KC = D // P        # 8 feature chunks
SC = S // P        # 8 key chunks
FC = F // P        # 32 ffn chunks
TQ = 512           # query tokens per core
NCORES = 8

# per-partition parameter table layout (each param chunk = one column)
PP_BASE = {}
_off = 0
for _name, _n in (("bq", 24), ("bk", 24), ("bo", 24), ("b1", FC), ("b2", KC),
                  ("g", 32), ("b", 32)):
    PP_BASE[_name] = _off
    _off += _n
PP_COLS = _off

_cache = {}

# ---- packed input blob layout (element offsets, bf16) ----
# One flat bf16 tensor per core holds all weights + activations + masks so
# the SPMD executable has a single input parameter (per-exec dispatch cost
# scales with the argument count).
_BLOB = {}
_off_e = 0
def _blob_reg(name, n_elems):
    global _off_e
    _BLOB[name] = (_off_e, n_elems)
    _off_e += n_elems

for _a in range(3):
    _blob_reg(f"wq{_a}", D * D)
    _blob_reg(f"wk{_a}", D * D)
    _blob_reg(f"wv{_a}", D * D)
    _blob_reg(f"wo{_a}", D * D)
_blob_reg("fc1", D * F)
_blob_reg("fc2", F * D)
_blob_reg("x_t", D * S)
_blob_reg("enc_t", D * TQ)
_blob_reg("gra_t", D * TQ)
_blob_reg("m_self", S * TQ)
_blob_reg("m_enc", S * TQ)
_blob_reg("m_gra", S * TQ)
BLOB_ELEMS = _off_e


def build(flags):
    """Builds the per-core Bass program. flags control which (generically
    correct) bias/affine paths get emitted; for the reference inputs all
    biases are zero and gammas are one, so these stay off."""
    nc = bacc.Bacc()

    dp = nc.declare_dram_parameter
    blob = dp("blob", [BLOB_ELEMS], BF16, isOutput=False)

    def reg(name):
        o, n = _BLOB[name]
        return blob[o:o + n]

    def panels4(name, n_chunks, k_chunks):
        # weight panels, partition-major: [n_chunk, partition, k_chunk, m]
        return reg(name).rearrange("(n p k m) -> n p k m",
                                   n=n_chunks, p=P, k=k_chunks)

    x_t = reg("x_t").rearrange("(kc p t) -> p kc t", p=P, t=S)
    enc_t = reg("enc_t").rearrange("(kc p t) -> p kc t", p=P, t=TQ)
    gra_t = reg("gra_t").rearrange("(kc p t) -> p kc t", p=P, t=TQ)
    m_self = reg("m_self").rearrange("(sc p t) -> p sc t", p=P, t=TQ)
    m_enc = reg("m_enc").rearrange("(sc p t) -> p sc t", p=P, t=TQ)
    m_gra = reg("m_gra").rearrange("(sc p t) -> p sc t", p=P, t=TQ)
    wq = [panels4(f"wq{a}", KC, KC) for a in range(3)]
    wk = [panels4(f"wk{a}", KC, KC) for a in range(3)]
    wv = [reg(f"wv{a}").rearrange("(kc p n) -> p kc n", p=P, n=D)
          for a in range(3)]
    wo = [panels4(f"wo{a}", KC, KC) for a in range(3)]
    fc1 = panels4("fc1", FC, KC)
    fc2 = panels4("fc2", KC, FC)
    need_pp = any(flags.values())
    pp = dp("pp", [P, PP_COLS], F32, isOutput=False) if need_pp else None
    bv = dp("bv", [3, D], BF16, isOutput=False) if flags["bv"] else None
    out_t = dp("out_t", [D, TQ], F32, isOutput=True)
    # DRAM bounce buffers for the pairwise K/V AllGather: each core projects
    # K/V only for its own 512 source tokens, then the core pair exchanges
    # halves (AllGather concatenates in core order = natural token order).
    kvin = [nc.dram_tensor(f"kv_in{a}", [2 * D * TQ], BF16) for a in (1, 2)]
    kvout = [nc.dram_tensor(f"kv_out{a}", [2, 2 * D * TQ], BF16)
             for a in (1, 2)]
    PAIRS = [[0, 1], [2, 3], [4, 5], [6, 7]]

    with TileContext(nc) as tc, ExitStack() as ctx:
        const = ctx.enter_context(tc.tile_pool(name="const", bufs=1))
        persist = ctx.enter_context(tc.tile_pool(name="persist", bufs=1))
        srcp = ctx.enter_context(tc.tile_pool(name="srcp", bufs=1))
        maskp = ctx.enter_context(tc.tile_pool(name="maskp", bufs=2))
        kvp = ctx.enter_context(tc.tile_pool(name="kvp", bufs=1))
        wpool = ctx.enter_context(tc.tile_pool(name="wpool", bufs=3))
        wvpool = ctx.enter_context(tc.tile_pool(name="wvpool", bufs=1))
        kvbp = ctx.enter_context(tc.tile_pool(name="kvbp", bufs=2))
        epool = ctx.enter_context(tc.tile_pool(name="epool", bufs=3))
        esump = ctx.enter_context(tc.tile_pool(name="esump", bufs=2))
        attnp = ctx.enter_context(tc.tile_pool(name="attnp", bufs=1))
        tmpp = ctx.enter_context(tc.tile_pool(name="tmpp", bufs=2))
        lntp = ctx.enter_context(tc.tile_pool(name="lntp", bufs=2))
        psum = ctx.enter_context(tc.tile_pool(name="psum", bufs=2, space="PSUM"))
        psum1 = ctx.enter_context(tc.tile_pool(name="psum1", bufs=1, space="PSUM"))

        ones = const.tile([P, P], BF16, tag="ones")
        nc.vector.memset(ones, 1.0)
        onesM = const.tile([P, P], BF16, tag="onesM")
        nc.vector.memset(onesM, 1.0 / D)
        epsc = const.tile([P, 1], F32, tag="epsc")
        nc.vector.memset(epsc, EPS)
        if need_pp:
            pp_sb = const.tile([P, PP_COLS], F32, tag="pp")
            nc.sync.dma_start(pp_sb, pp[:])
        if flags["bv"]:
            bv_sb = const.tile([3, D], BF16, tag="bv")
            nc.sync.dma_start(bv_sb, bv[:])

        def ppc(name, c):
            base = PP_BASE[name]
            return pp_sb[:, base + c: base + c + 1]

        # ---------- helpers ----------
        def proj_fm(dst, w_dram, rhs_sb, n_chunks, k_chunks, Tt,
                    bias=None, relu=False, evict=None, kq_split=1):
            """Feature-major projection: dst[:, ncn, tslice] = panel.T @ rhs.
            kq_split loads the k-dim weight panel in pieces (for big k)."""
            kq_n = k_chunks // kq_split
            assert kq_split == 1 or Tt == 512
            for ncn in range(n_chunks):
                shared_wp = None
                if kq_split == 1:
                    shared_wp = wpool.tile([P, k_chunks, P], BF16, tag="wp",
                                           name="wp")
                    nc.sync.dma_start(shared_wp, w_dram[ncn])
                for tn in range(Tt // 512):
                    ps = psum.tile([P, 512], F32, tag="proj", name="ps_proj")
                    for kq in range(kq_split):
                        if shared_wp is not None:
                            wp = shared_wp
                        else:
                            wp = wpool.tile([P, kq_n, P], BF16, tag="wp",
                                            name="wp")
                            nc.sync.dma_start(
                                wp, w_dram[ncn][:, kq * kq_n:(kq + 1) * kq_n]
                            )
                        for kl in range(kq_n):
                            kc = kq * kq_n + kl
                            nc.tensor.matmul(
                                ps, wp[:, kl],
                                rhs_sb[:, kc, tn * 512:(tn + 1) * 512],
                                start=(kc == 0), stop=(kc == k_chunks - 1),
                            )
                    d = dst[:, ncn, tn * 512:(tn + 1) * 512]
                    if evict is not None:
                        evict(ncn, ps, d)
                    elif relu:
                        if bias is not None:
                            nc.scalar.activation(d, ps, AF.Relu, bias=bias(ncn))
                        else:
                            nc.scalar.activation(d, ps, AF.Relu)
                    elif bias is not None:
                        nc.vector.tensor_scalar(d, ps, bias(ncn), None, ALU.add)
                    else:
                        nc.scalar.copy(d, ps)

        def proj_tm(v_sb, wv_dram, src_sb, a):
            """Token-major V projection: v_sb[:, sc, :]."""
            wv_sb = wvpool.tile([P, KC, D], BF16, tag="wv", name="wv_sb")
            nc.sync.dma_start(wv_sb, wv_dram)
            use_bias = flags["bv"]
            for sc in range(SC):
                for n2 in range(2):
                    ps = psum.tile([P, 512], F32, tag="proj", name="ps_v")
                    for kc in range(KC):
                        nc.tensor.matmul(
                            ps,
                            src_sb[:, kc, sc * P:(sc + 1) * P],
                            wv_sb[:, kc, n2 * 512:(n2 + 1) * 512],
                            start=(kc == 0),
                            stop=(kc == KC - 1 and not use_bias),
                        )
                    if use_bias:
                        nc.tensor.matmul(
                            ps, ones[0:1, :],
                            bv_sb[a:a + 1, n2 * 512:(n2 + 1) * 512],
                            start=False, stop=True,
                        )
                    nc.scalar.copy(v_sb[:, sc, n2 * 512:(n2 + 1) * 512], ps)

        def stage_groups(a, s_sb):
            """Emit-closures that project attention a's K/V for this core's
            OWN 512 source tokens into the AllGather bounce buffer, group by
            group (each closure = one PSUM group). The final closure issues
            the pairwise AllGather. Staging evictions go through DVE so they
            don't queue behind the exp storm on ACT during attention."""
            groups = []
            i = a - 1
            dst_k = kvin[i][0:D * TQ].rearrange("(nc p t) -> nc p t",
                                                p=P, t=TQ)
            dst_v = kvin[i][D * TQ:2 * D * TQ].rearrange(
                "(sc p n) -> sc p n", sc=SC // 2, p=P)
            use_kb = flags["bk"]
            use_vb = flags["bv"]
            panel_cache = {}
            wv_holder = {}

            def k_closure(ncn):
                def run():
                    wp = wpool.tile([P, KC, P], BF16, tag="wp", name="wp")
                    nc.sync.dma_start(wp, wk[a][ncn])
                    ps = psum.tile([P, 512], F32, tag="proj", name="ps_kst")
                    for kc in range(KC):
                        nc.tensor.matmul(
                            ps, wp[:, kc], s_sb[:, kc],
                            start=(kc == 0), stop=(kc == KC - 1),
                        )
                    bt = kvbp.tile([P, 512], BF16, tag="kvb", name="kvb")
                    if use_kb:
                        nc.vector.tensor_scalar(bt, ps, ppc("bk", a * KC + ncn),
                                                None, ALU.add)
                    else:
                        nc.vector.tensor_copy(bt, ps)
                    nc.sync.dma_start(dst_k[ncn], bt)

                return run

            for ncn in range(KC):
                groups.append(k_closure(ncn))

            def v_closure(sc, n2):
                def run():
                    if "wv" not in wv_holder:
                        wv_sb = wvpool.tile([P, KC, D], BF16, tag="wv",
                                            name="wv_sb")
                        nc.sync.dma_start(wv_sb, wv[a])
                        wv_holder["wv"] = wv_sb
                    wv_sb = wv_holder["wv"]
                    ps = psum.tile([P, 512], F32, tag="proj", name="ps_vst")
                    for kc in range(KC):
                        nc.tensor.matmul(
                            ps,
                            s_sb[:, kc, sc * P:(sc + 1) * P],
                            wv_sb[:, kc, n2 * 512:(n2 + 1) * 512],
                            start=(kc == 0),
                            stop=(kc == KC - 1 and not use_vb),
                        )
                    if use_vb:
                        nc.tensor.matmul(
                            ps, ones[0:1, :],
                            bv_sb[a:a + 1, n2 * 512:(n2 + 1) * 512],
                            start=False, stop=True,
                        )
                    bt = kvbp.tile([P, 512], BF16, tag="kvb", name="kvb2")
                    nc.vector.tensor_copy(bt, ps)
                    nc.sync.dma_start(dst_v[sc][:, n2 * 512:(n2 + 1) * 512], bt)

                return run

            for sc in range(SC // 2):
                for n2 in range(2):
                    groups.append(v_closure(sc, n2))

            def ag_closure():
                nc.gpsimd.collective_compute(
                    "AllGather", ALU.bypass,
                    replica_groups=PAIRS,
                    ins=[kvin[i][:].opt()], outs=[kvout[i][:].opt()],
                )

            groups.append(ag_closure)
            return groups

        def attention(q_sb, k_sb, v_sb, em_sb, attn_out, filler=None):
            """attn_out[:, hp, :] = softmax-attention, heads in pairs.
            em_sb holds exp(mask); the pair's two K=64 score matmuls are
            emitted back-to-back so they overlap in different PE row groups."""
            for hp in range(NH // 2):
                ps_z = psum1.tile([P, 512], F32, tag="z", name="ps_z")
                ps_o = psum1.tile([P, 512], F32, tag="o", name="ps_o")
                kc_h = hp  # heads 2hp (rows 0:64) and 2hp+1 (rows 64:128)
                esum = esump.tile([P, 2, TQ], BF16, tag="esum", name="esum")
                for sc in range(SC):
                    # both heads' scores in one 2-bank PSUM tile
                    ps = psum.tile([P, 2, 512], F32, tag="scores", name="ps_sc")
                    for j in range(2):
                        off = j * 64
                        nc.tensor.matmul(
                            ps[:, j],
                            k_sb[off:off + DH, kc_h, sc * P:(sc + 1) * P],
                            q_sb[off:off + DH, kc_h],
                            start=True, stop=True,
                        )
                    e_sb = epool.tile([P, 2, TQ], BF16, tag="e", name="e_sb")
                    nc.scalar.activation(e_sb, ps, AF.Exp)
                    nc.vector.tensor_mul(
                        e_sb, e_sb,
                        em_sb[:, sc:sc + 1, :].to_broadcast((P, 2, TQ)),
                    )
                    if sc == 0:
                        nc.vector.tensor_copy(esum, e_sb)
                    else:
                        nc.vector.tensor_add(esum, esum, e_sb)
                    for j in range(2):
                        h = 2 * hp + j
                        nc.tensor.matmul(
                            ps_o[j * 64:(j + 1) * 64],
                            v_sb[:, sc, h * DH:(h + 1) * DH], e_sb[:, j],
                            start=(sc == 0), stop=(sc == SC - 1),
                            tile_position=(0, j * 64),
                        )
                for j in range(2):
                    nc.tensor.matmul(
                        ps_z[j * 64:(j + 1) * 64], ones[:, :64], esum[:, j],
                        start=True, stop=True,
                        tile_position=(0, j * 64),
                    )
                rz = tmpp.tile([P, TQ], F32, tag="rz", name="rz")
                nc.vector.reciprocal_approx_fast(rz, ps_z)
                nc.vector.tensor_mul(attn_out[:, hp], ps_o, rz)
                if filler is not None:
                    filler(hp)

        def layer_norm(dst_fn, z_sb, li, post=None):
            """dst_fn(kc) <- (z - mu) * rstd [* g + b], feature-major.
            post(kc, ap) runs after each chunk is written."""
            ps_m = psum1.tile([P, 512], F32, tag="z", name="ps_m")
            for kc in range(KC):
                nc.tensor.matmul(ps_m, onesM, z_sb[:, kc],
                                 start=(kc == 0), stop=(kc == KC - 1))
            ps_s = psum1.tile([P, 512], F32, tag="o", name="ps_s")
            for kc in range(KC):
                zsq = lntp.tile([P, TQ], BF16, tag="zsq", name="zsq")
                nc.vector.tensor_mul(zsq, z_sb[:, kc], z_sb[:, kc])
                nc.tensor.matmul(ps_s, onesM, zsq,
                                 start=(kc == 0), stop=(kc == KC - 1))
            musq = tmpp.tile([P, TQ], F32, tag="stat", name="musq")
            nc.scalar.square(musq, ps_m)
            var = tmpp.tile([P, TQ], F32, tag="stat", name="var")
            nc.vector.tensor_sub(var, ps_s, musq)
            sd = tmpp.tile([P, TQ], F32, tag="stat", name="sd")
            nc.scalar.activation(sd, var, AF.Sqrt, bias=epsc)
            rstd = tmpp.tile([P, TQ], F32, tag="stat", name="rstd")
            nc.vector.reciprocal_approx_fast(rstd, sd)
            for kc in range(KC):
                t1 = lntp.tile([P, TQ], F32, tag="lnt", name="lnt")
                nc.vector.tensor_sub(t1, z_sb[:, kc], ps_m)
                d = dst_fn(kc)
                if flags["ln_affine"]:
                    t2 = lntp.tile([P, TQ], F32, tag="lnt2", name="lnt2")
                    nc.vector.tensor_mul(t2, t1, rstd)
                    nc.vector.tensor_scalar(
                        d, t2, ppc("g", li * KC + kc), ppc("b", li * KC + kc),
                        ALU.mult, ALU.add,
                    )
                else:
                    nc.vector.tensor_mul(d, t1, rstd)
                if post is not None:
                    post(kc, d)

        # ---------- self attention ----------
        x_sb = srcp.tile([P, KC, S], BF16, tag="srcx", name="x_sb")
        nc.sync.dma_start(x_sb, x_t)
        ms_sb = maskp.tile([P, SC, TQ], BF16, tag="mask", name="ms_sb")
        nc.sync.dma_start(ms_sb, m_self)
        em0_sb = ms_sb  # exp(mask) computed in place, chunk by chunk
        for sc in range(SC):
            nc.scalar.activation(em0_sb[:, sc], ms_sb[:, sc], AF.Exp)
        xq_view = x_sb[:, :, 0:TQ]  # own queries are the first TQ columns

        q_sb = kvp.tile([P, KC, TQ], BF16, tag="Q", name="q0")
        proj_fm(q_sb, wq[0], xq_view, KC, KC, TQ,
                bias=(lambda c: ppc("bq", c)) if flags["bq"] else None)
        k_sb = kvp.tile([P, KC, S], BF16, tag="K", name="k0")
        proj_fm(k_sb, wk[0], x_sb, KC, KC, S,
                bias=(lambda c: ppc("bk", c)) if flags["bk"] else None)
        v_sb = kvp.tile([P, SC, D], BF16, tag="V", name="v0")
        proj_tm(v_sb, wv[0], x_sb, 0)

        # cross/graph K/V (own token half only) get projected into the
        # AllGather bounce buffers; the PE work is interleaved (via fillers)
        # with the ACT-bound self-attention head loop, and each AllGather is
        # issued as soon as its half is staged.
        enc_sb = srcp.tile([P, KC, TQ], BF16, tag="srce", name="src1")
        nc.sync.dma_start(enc_sb, enc_t)
        gra_sb = srcp.tile([P, KC, TQ], BF16, tag="srcg", name="src2")
        nc.sync.dma_start(gra_sb, gra_t)
        all_groups = stage_groups(1, enc_sb) + stage_groups(2, gra_sb)

        def filler0(hp):
            n = len(all_groups)
            for g in all_groups[hp * n // 8:(hp + 1) * n // 8]:
                g()

        attn0 = attnp.tile([P, KC, TQ], BF16, tag="attn", name="attn0")
        attention(q_sb, k_sb, v_sb, em0_sb, attn0, filler=filler0)

        z1 = persist.tile([P, KC, TQ], BF16, tag="zres", name="z1")

        def evict_res0(ncn, ps, d):
            if flags["bo"]:
                t = tmpp.tile([P, TQ], F32, tag="evt", name="evt")
                nc.vector.tensor_scalar(t, ps, ppc("bo", ncn), None, ALU.add)
                nc.vector.tensor_add(d, t, x_sb[:, ncn, 0:TQ])
            else:
                nc.vector.tensor_add(d, ps, x_sb[:, ncn, 0:TQ])

        proj_fm(z1, wo[0], attn0, KC, KC, TQ, evict=evict_res0)
        h1 = persist.tile([P, KC, TQ], BF16, tag="h1")
        layer_norm(lambda kc: h1[:, kc], z1, 0)

        # ---------- cross + graph attention ----------
        h2 = persist.tile([P, KC, TQ], BF16, tag="h2")
        for a, m_d in ((1, m_enc), (2, m_gra)):
            m_sb = maskp.tile([P, SC, TQ], BF16, tag="mask", name=f"m{a}")
            nc.sync.dma_start(m_sb, m_d)
            em_sb = m_sb  # exp(mask) in place
            for sc in range(SC):
                nc.scalar.activation(em_sb[:, sc], m_sb[:, sc], AF.Exp)

            qa = kvp.tile([P, KC, TQ], BF16, tag="Q", name=f"q{a}")
            proj_fm(qa, wq[a], h1, KC, KC, TQ,
                    bias=(lambda c: ppc("bq", a * KC + c)) if flags["bq"] else None)
            ka = kvp.tile([P, KC, S], BF16, tag="K", name=f"k{a}")
            va = kvp.tile([P, SC, D], BF16, tag="V", name=f"v{a}")
            for sl in range(2):
                nc.sync.dma_start(
                    ka[:, :, sl * TQ:(sl + 1) * TQ],
                    kvout[a - 1][sl, 0:D * TQ].rearrange(
                        "(nc p t) -> p nc t", p=P, t=TQ))
                nc.sync.dma_start(
                    va[:, sl * (SC // 2):(sl + 1) * (SC // 2)],
                    kvout[a - 1][sl, D * TQ:2 * D * TQ].rearrange(
                        "(sc p n) -> p sc n", sc=SC // 2, p=P))

            attn_a = attnp.tile([P, KC, TQ], BF16, tag="attn", name=f"attn{a}")
            attention(qa, ka, va, em_sb, attn_a, filler=None)

            za = persist.tile([P, KC, TQ], BF16, tag="zres", name=f"za{a}")

            def evict_o(ncn, ps, d, a=a):
                if flags["bo"]:
                    nc.vector.tensor_scalar(d, ps, ppc("bo", a * KC + ncn),
                                            None, ALU.add)
                else:
                    nc.scalar.copy(d, ps)

            proj_fm(za, wo[a], attn_a, KC, KC, TQ, evict=evict_o)
            base = h1 if a == 1 else h2

            def post_add(kc, ap, base=base):
                nc.vector.tensor_add(h2[:, kc], base[:, kc], ap)

            layer_norm(
                lambda kc: lntp.tile([P, TQ], BF16, tag="lnc", name="lnc"),
                za, a, post=post_add,
            )

        # ---------- FFN ----------
        r_sb = persist.tile([P, FC, TQ], BF16, tag="r")
        proj_fm(r_sb, fc1, h2, FC, KC, TQ,
                bias=(lambda c: ppc("b1", c)) if flags["b1"] else None,
                relu=True)

        z3 = persist.tile([P, KC, TQ], BF16, tag="zres", name="z3")

        def evict_fc2(ncn, ps, d):
            if flags["b2"]:
                t = tmpp.tile([P, TQ], F32, tag="evt", name="evt2")
                nc.vector.tensor_scalar(t, ps, ppc("b2", ncn), None, ALU.add)
                nc.vector.tensor_add(d, t, h2[:, ncn])
            else:
                nc.vector.tensor_add(d, ps, h2[:, ncn])

        proj_fm(z3, fc2, r_sb, KC, FC, TQ, evict=evict_fc2, kq_split=4)

        out_r = out_t.rearrange("(kc p) t -> kc p t", p=P)
        layer_norm(
            lambda kc: lntp.tile([P, TQ], F32, tag="ochunk", name="ochunk"),
            z3, 3,
            post=lambda kc, ap: nc.sync.dma_start(out_r[kc], ap),
        )

    nc.finalize()
    return nc


def _pp_table(b_q, b_k, b_o, fc1_b, fc2_b, ln_g, ln_b):
    t = np.zeros((P, PP_COLS), np.float32)

    def put(name, vec):
        v = np.asarray(vec, np.float32).reshape(-1, P).T  # [128, n]
        t[:, PP_BASE[name]: PP_BASE[name] + v.shape[1]] = v

    put("bq", b_q.reshape(-1))
    put("bk", b_k.reshape(-1))
    put("bo", b_o.reshape(-1))
    put("b1", fc1_b)
    put("b2", fc2_b)
    put("g", ln_g.reshape(-1))
    put("b", ln_b.reshape(-1))
    return t


def _panels(w):
    """[Din, Dout] -> [Dout//128, 128(p), Din//128, 128(m)] partition-major
    column panels (each SBUF partition line is one contiguous run)."""
    din, dout = w.shape
    return np.ascontiguousarray(
        w.reshape(din // P, P, dout // P, P).transpose(2, 1, 0, 3)
    )


def _bf(a):
    return np.ascontiguousarray(np.asarray(a)).astype(ml_dtypes.bfloat16)


def _blob_put(blob, name, arr):
    o, n = _BLOB[name]
    a = np.ascontiguousarray(arr).reshape(-1)
    assert a.size == n, (name, a.size, n)
    blob[o:o + n] = a


def prepare(inputs):
    """Host-side prep: returns (flags, in_maps)."""
    ii = {k: np.asarray(v, np.float32) for k, v in inputs.items()}
    flags = {
        "bq": bool(np.any(ii["b_q"])),
        "bk": bool(np.any(ii["b_k"])),
        "bv": bool(np.any(ii["b_v"])),
        "bo": bool(np.any(ii["b_o"])),
        "b1": bool(np.any(ii["fc1_b"])),
        "b2": bool(np.any(ii["fc2_b"])),
        "ln_affine": bool(np.any(ii["ln_b"])
                          or not np.allclose(ii["ln_g"], 1.0)),
    }
    need_pp = any(flags.values())

    base = np.zeros(BLOB_ELEMS, ml_dtypes.bfloat16)
    _blob_put(base, "fc1", _bf(_panels(ii["fc1_w"])))
    _blob_put(base, "fc2", _bf(_panels(ii["fc2_w"])))
    for a in range(3):
        _blob_put(base, f"wq{a}", _bf(_panels(ii["W_q"][a] * SCALE)))
        _blob_put(base, f"wk{a}", _bf(_panels(ii["W_k"][a])))
        _blob_put(base, f"wv{a}", _bf(ii["W_v"][a]))
        _blob_put(base, f"wo{a}", _bf(_panels(ii["W_o"][a])))

    hid, enc, gra = (ii["hidden_states"], ii["enc_hidden_states"],
                     ii["graph_hidden_states"])
    msk = [ii["dec_self_mask"], ii["enc_dec_mask"], ii["graph_dec_mask"]]

    extra = {}
    if need_pp:
        extra["pp"] = _pp_table(ii["b_q"] * SCALE, ii["b_k"], ii["b_o"],
                                ii["fc1_b"], ii["fc2_b"], ii["ln_g"],
                                ii["ln_b"])
    if flags["bv"]:
        extra["bv"] = _bf(ii["b_v"])

    in_maps = []
    for c in range(NCORES):
        b, half = divmod(c, 2)
        r0 = half * TQ
        perm = np.r_[r0:S, 0:r0]  # own tokens first (self-attn key axis)
        blob = base.copy()
        _blob_put(blob, "x_t", _bf(hid[b].T[:, perm]))
        _blob_put(blob, "enc_t", _bf(enc[b].T[:, r0:r0 + TQ]))
        _blob_put(blob, "gra_t", _bf(gra[b].T[:, r0:r0 + TQ]))
        _blob_put(blob, "m_self", _bf(msk[0][b, 0].T[perm][:, r0:r0 + TQ]))
        _blob_put(blob, "m_enc", _bf(msk[1][b, 0].T[:, r0:r0 + TQ]))
        _blob_put(blob, "m_gra", _bf(msk[2][b, 0].T[:, r0:r0 + TQ]))
        in_maps.append({"blob": blob, **extra})
    return flags, in_maps


def get_program(flags):
    key = tuple(sorted(flags.items()))
    if key not in _cache:
        _cache[key] = build(flags)
    return _cache[key]


def gather(results):
    out = np.zeros((B, T, D), np.float32)
    for c in range(NCORES):
        b, half = divmod(c, 2)
        r0 = half * TQ
        out[b, r0:r0 + TQ, :] = results[c]["out_t"].T
    return out


def kernel(**inputs) -> np.ndarray:
    from concourse.bass_utils import run_bass_kernel_spmd

    flags, in_maps = prepare(inputs)
    nc = get_program(flags)
    res = run_bass_kernel_spmd(nc, in_maps, list(range(NCORES)))
    return gather(res.results)



# revision 11
# speedup vs baseline: 1.1017x; 1.1017x over previous
"""Trainium2 Bass kernel for nn_DecoderLayer_72327249264859.

Decoder layer: self-attn + (cross-attn || graph-attn) + FFN, each with
residual + layernorm. B=4, T=S=1024, D=1024, 16 heads, ffn=4096.

Sharding: pure data-parallel over query tokens. Core c handles batch
element b = c // 2, query rows (c % 2)*512 .. +512. Each core computes
full-length K/V for its batch element (2x duplicated K/V work, but zero
cross-core communication). The host permutes the self-attention key axis
so each core's own query tokens are the first 512 columns of x_t
(attention is invariant to key order as long as the mask rows are
permuted identically), so the same SPMD program works on every core.

Layout strategy: activations are kept *feature-major* in SBUF
([feature-dim on partitions, tokens on free]):
  - feature-major projections: lhsT = W column-panel [Din, 128] (natural),
    rhs = X_t [d, t] -> out [n, t]; no transposes anywhere;
  - token-major outputs (V for probs@V) use lhsT = X_t chunk (activations
    stationary) and rhs = W natural [Din, Dout];
  - scores are computed transposed ([s part, t free]): lhsT = K_t slice,
    rhs = Q_t slice (contraction over dh=64);
  - the additive mask (host-pre-transposed) is accumulated into the scores
    PSUM with an identity-matrix matmul;
  - softmax-Z (partition reduction) is an all-ones matmul that lands
    replicated across partitions in PSUM; 1/Z via fast DVE reciprocal;
  - layernorm mean / mean-of-squares via (1/D)-ones matmuls, same trick.
All matmuls bf16 with fp32 PSUM accumulation; heads processed in pairs
sharing PSUM tiles (partition halves).
"""

import sys

if "/opt/trn_rl_repo" not in sys.path:
    sys.path.insert(0, "/opt/trn_rl_repo")

import numpy as np
import ml_dtypes
from contextlib import ExitStack

import concourse.bacc as bacc
import concourse.mybir as mybir
from concourse.tile import TileContext

BF16 = mybir.dt.bfloat16
F32 = mybir.dt.float32
AF = mybir.ActivationFunctionType
ALU = mybir.AluOpType

B, T, S, D = 4, 1024, 1024, 1024
NH, DH = 16, 64
F = 4 * D
SCALE = DH**-0.5
EPS = 1e-5
P = 128
KC = D // P        # 8 feature chunks
SC = S // P        # 8 key chunks
FC = F // P        # 32 ffn chunks
TQ = 512           # query tokens per core
NCORES = 8

# per-partition parameter table layout (each param chunk = one column)
PP_BASE = {}
_off = 0
for _name, _n in (("bq", 24), ("bk", 24), ("bo", 24), ("b1", FC), ("b2", KC),
                  ("g", 32), ("b", 32)):
    PP_BASE[_name] = _off
    _off += _n
PP_COLS = _off

_cache = {}

# ---- packed input blob layout (element offsets, bf16) ----
# One flat bf16 tensor per core holds all weights + activations + masks so
# the SPMD executable has a single input parameter (per-exec dispatch cost
# scales with the argument count).
_BLOB = {}
_off_e = 0
def _blob_reg(name, n_elems):
    global _off_e
    _BLOB[name] = (_off_e, n_elems)
    _off_e += n_elems

for _a in range(3):
    _blob_reg(f"wq{_a}", D * D)
    _blob_reg(f"wk{_a}", D * D)
    _blob_reg(f"wv{_a}", D * D)
    _blob_reg(f"wo{_a}", D * D)
_blob_reg("fc1", D * F)
_blob_reg("fc2", F * D)
_blob_reg("x_t", D * S)
_blob_reg("enc_t", D * TQ)
_blob_reg("gra_t", D * TQ)
_blob_reg("m_self", S * TQ)
_blob_reg("m_enc", S * TQ)
_blob_reg("m_gra", S * TQ)
BLOB_ELEMS = _off_e


def build(flags):
    """Builds the per-core Bass program. flags control which (generically
    correct) bias/affine paths get emitted; for the reference inputs all
    biases are zero and gammas are one, so these stay off."""
    nc = bacc.Bacc()

    dp = nc.declare_dram_parameter
    blob = dp("blob", [BLOB_ELEMS], BF16, isOutput=False)

    def reg(name):
        o, n = _BLOB[name]
        return blob[o:o + n]

    def panels4(name, n_chunks, k_chunks):
        # weight panels, partition-major: [n_chunk, partition, k_chunk, m]
        return reg(name).rearrange("(n p k m) -> n p k m",
                                   n=n_chunks, p=P, k=k_chunks)

    x_t = reg("x_t").rearrange("(kc p t) -> p kc t", p=P, t=S)
    enc_t = reg("enc_t").rearrange("(kc p t) -> p kc t", p=P, t=TQ)
    gra_t = reg("gra_t").rearrange("(kc p t) -> p kc t", p=P, t=TQ)
    m_self = reg("m_self").rearrange("(sc p t) -> p sc t", p=P, t=TQ)
    m_enc = reg("m_enc").rearrange("(sc p t) -> p sc t", p=P, t=TQ)
    m_gra = reg("m_gra").rearrange("(sc p t) -> p sc t", p=P, t=TQ)
    wq = [panels4(f"wq{a}", KC, KC) for a in range(3)]
    wk = [panels4(f"wk{a}", KC, KC) for a in range(3)]
    wv = [reg(f"wv{a}").rearrange("(kc p n) -> p kc n", p=P, n=D)
          for a in range(3)]
    wo = [panels4(f"wo{a}", KC, KC) for a in range(3)]
    fc1 = panels4("fc1", FC, KC)
    fc2 = panels4("fc2", KC, FC)
    need_pp = any(flags.values())
    pp = dp("pp", [P, PP_COLS], F32, isOutput=False) if need_pp else None
    bv = dp("bv", [3, D], BF16, isOutput=False) if flags["bv"] else None
    out_t = dp("out_t", [D, TQ], F32, isOutput=True)
    # DRAM bounce buffers for the pairwise K/V AllGather: each core projects
    # K/V only for its own 512 source tokens, then the core pair exchanges
    # halves (AllGather concatenates in core order = natural token order).
    kvin_all = nc.dram_tensor("kv_in", [2 * 2 * D * TQ], BF16)
    kvin = [kvin_all[0:2 * D * TQ], kvin_all[2 * D * TQ:4 * D * TQ]]
    kvout_all = nc.dram_tensor("kv_out", [2, 2 * 2 * D * TQ], BF16)
    kvout = [kvout_all[:, 0:2 * D * TQ], kvout_all[:, 2 * D * TQ:4 * D * TQ]]
    PAIRS = [[0, 1], [2, 3], [4, 5], [6, 7]]

    with TileContext(nc) as tc, ExitStack() as ctx:
        const = ctx.enter_context(tc.tile_pool(name="const", bufs=1))
        persist = ctx.enter_context(tc.tile_pool(name="persist", bufs=1))
        srcp = ctx.enter_context(tc.tile_pool(name="srcp", bufs=1))
        maskp = ctx.enter_context(tc.tile_pool(name="maskp", bufs=2))
        kvp = ctx.enter_context(tc.tile_pool(name="kvp", bufs=1))
        wpool = ctx.enter_context(tc.tile_pool(name="wpool", bufs=3))
        wvpool = ctx.enter_context(tc.tile_pool(name="wvpool", bufs=1))
        kvbp = ctx.enter_context(tc.tile_pool(name="kvbp", bufs=2))
        epool = ctx.enter_context(tc.tile_pool(name="epool", bufs=3))
        esump = ctx.enter_context(tc.tile_pool(name="esump", bufs=2))
        attnp = ctx.enter_context(tc.tile_pool(name="attnp", bufs=1))
        tmpp = ctx.enter_context(tc.tile_pool(name="tmpp", bufs=2))
        lntp = ctx.enter_context(tc.tile_pool(name="lntp", bufs=2))
        psum = ctx.enter_context(tc.tile_pool(name="psum", bufs=2, space="PSUM"))
        psum1 = ctx.enter_context(tc.tile_pool(name="psum1", bufs=1, space="PSUM"))

        ones = const.tile([P, P], BF16, tag="ones")
        nc.vector.memset(ones, 1.0)
        onesM = const.tile([P, P], BF16, tag="onesM")
        nc.vector.memset(onesM, 1.0 / D)
        epsc = const.tile([P, 1], F32, tag="epsc")
        nc.vector.memset(epsc, EPS)
        if need_pp:
            pp_sb = const.tile([P, PP_COLS], F32, tag="pp")
            nc.sync.dma_start(pp_sb, pp[:])
        if flags["bv"]:
            bv_sb = const.tile([3, D], BF16, tag="bv")
            nc.sync.dma_start(bv_sb, bv[:])

        def ppc(name, c):
            base = PP_BASE[name]
            return pp_sb[:, base + c: base + c + 1]

        # ---------- helpers ----------
        def proj_fm(dst, w_dram, rhs_sb, n_chunks, k_chunks, Tt,
                    bias=None, relu=False, evict=None, kq_split=1):
            """Feature-major projection: dst[:, ncn, tslice] = panel.T @ rhs.
            kq_split loads the k-dim weight panel in pieces (for big k)."""
            kq_n = k_chunks // kq_split
            assert kq_split == 1 or Tt == 512
            for ncn in range(n_chunks):
                shared_wp = None
                if kq_split == 1:
                    shared_wp = wpool.tile([P, k_chunks, P], BF16, tag="wp",
                                           name="wp")
                    nc.sync.dma_start(shared_wp, w_dram[ncn])
                for tn in range(Tt // 512):
                    ps = psum.tile([P, 512], F32, tag="proj", name="ps_proj")
                    for kq in range(kq_split):
                        if shared_wp is not None:
                            wp = shared_wp
                        else:
                            wp = wpool.tile([P, kq_n, P], BF16, tag="wp",
                                            name="wp")
                            nc.sync.dma_start(
                                wp, w_dram[ncn][:, kq * kq_n:(kq + 1) * kq_n]
                            )
                        for kl in range(kq_n):
                            kc = kq * kq_n + kl
                            nc.tensor.matmul(
                                ps, wp[:, kl],
                                rhs_sb[:, kc, tn * 512:(tn + 1) * 512],
                                start=(kc == 0), stop=(kc == k_chunks - 1),
                            )
                    d = dst[:, ncn, tn * 512:(tn + 1) * 512]
                    if evict is not None:
                        evict(ncn, ps, d)
                    elif relu:
                        if bias is not None:
                            nc.scalar.activation(d, ps, AF.Relu, bias=bias(ncn))
                        else:
                            nc.scalar.activation(d, ps, AF.Relu)
                    elif bias is not None:
                        nc.vector.tensor_scalar(d, ps, bias(ncn), None, ALU.add)
                    else:
                        nc.scalar.copy(d, ps)

        def proj_tm(v_sb, wv_dram, src_sb, a):
            """Token-major V projection: v_sb[:, sc, :]."""
            wv_sb = wvpool.tile([P, KC, D], BF16, tag="wv", name="wv_sb")
            nc.sync.dma_start(wv_sb, wv_dram)
            use_bias = flags["bv"]
            for sc in range(SC):
                for n2 in range(2):
                    ps = psum.tile([P, 512], F32, tag="proj", name="ps_v")
                    for kc in range(KC):
                        nc.tensor.matmul(
                            ps,
                            src_sb[:, kc, sc * P:(sc + 1) * P],
                            wv_sb[:, kc, n2 * 512:(n2 + 1) * 512],
                            start=(kc == 0),
                            stop=(kc == KC - 1 and not use_bias),
                        )
                    if use_bias:
                        nc.tensor.matmul(
                            ps, ones[0:1, :],
                            bv_sb[a:a + 1, n2 * 512:(n2 + 1) * 512],
                            start=False, stop=True,
                        )
                    nc.scalar.copy(v_sb[:, sc, n2 * 512:(n2 + 1) * 512], ps)

        def stage_groups(a, s_sb):
            """Emit-closures that project attention a's K/V for this core's
            OWN 512 source tokens into the AllGather bounce buffer, group by
            group (each closure = one PSUM group). The final closure issues
            the pairwise AllGather. Staging evictions go through DVE so they
            don't queue behind the exp storm on ACT during attention."""
            groups = []
            i = a - 1
            base_e = i * 2 * D * TQ
            dst_k = kvin_all[base_e:base_e + D * TQ].rearrange(
                "(nc p t) -> nc p t", p=P, t=TQ)
            dst_v = kvin_all[base_e + D * TQ:base_e + 2 * D * TQ].rearrange(
                "(sc p n) -> sc p n", sc=SC // 2, p=P)
            use_kb = flags["bk"]
            use_vb = flags["bv"]
            panel_cache = {}
            wv_holder = {}

            def k_closure(ncn):
                def run():
                    wp = wpool.tile([P, KC, P], BF16, tag="wp", name="wp")
                    nc.sync.dma_start(wp, wk[a][ncn])
                    ps = psum.tile([P, 512], F32, tag="proj", name="ps_kst")
                    for kc in range(KC):
                        nc.tensor.matmul(
                            ps, wp[:, kc], s_sb[:, kc],
                            start=(kc == 0), stop=(kc == KC - 1),
                        )
                    bt = kvbp.tile([P, 512], BF16, tag="kvb", name="kvb")
                    if use_kb:
                        nc.vector.tensor_scalar(bt, ps, ppc("bk", a * KC + ncn),
                                                None, ALU.add)
                    else:
                        nc.vector.tensor_copy(bt, ps)
                    nc.sync.dma_start(dst_k[ncn], bt)

                return run

            for ncn in range(KC):
                groups.append(k_closure(ncn))

            def v_closure(sc, n2):
                def run():
                    if "wv" not in wv_holder:
                        wv_sb = wvpool.tile([P, KC, D], BF16, tag="wv",
                                            name="wv_sb")
                        nc.sync.dma_start(wv_sb, wv[a])
                        wv_holder["wv"] = wv_sb
                    wv_sb = wv_holder["wv"]
                    ps = psum.tile([P, 512], F32, tag="proj", name="ps_vst")
                    for kc in range(KC):
                        nc.tensor.matmul(
                            ps,
                            s_sb[:, kc, sc * P:(sc + 1) * P],
                            wv_sb[:, kc, n2 * 512:(n2 + 1) * 512],
                            start=(kc == 0),
                            stop=(kc == KC - 1 and not use_vb),
                        )
                    if use_vb:
                        nc.tensor.matmul(
                            ps, ones[0:1, :],
                            bv_sb[a:a + 1, n2 * 512:(n2 + 1) * 512],
                            start=False, stop=True,
                        )
                    bt = kvbp.tile([P, 512], BF16, tag="kvb", name="kvb2")
                    nc.vector.tensor_copy(bt, ps)
                    nc.sync.dma_start(dst_v[sc][:, n2 * 512:(n2 + 1) * 512], bt)

                return run

            for sc in range(SC // 2):
                for n2 in range(2):
                    groups.append(v_closure(sc, n2))

            def ag_closure():
                nc.gpsimd.collective_compute(
                    "AllGather", ALU.bypass,
                    replica_groups=PAIRS,
                    ins=[kvin_all[:].opt()], outs=[kvout_all[:].opt()],
                )

            if a == 2:
                groups.append(ag_closure)
            return groups

        def attention(q_sb, k_sb, v_sb, em_sb, attn_out, filler=None):
            """attn_out[:, hp, :] = softmax-attention, heads in pairs.
            em_sb holds exp(mask); the pair's two K=64 score matmuls are
            emitted back-to-back so they overlap in different PE row groups."""
            for hp in range(NH // 2):
                ps_z = psum1.tile([P, 512], F32, tag="z", name="ps_z")
                ps_o = psum1.tile([P, 512], F32, tag="o", name="ps_o")
                kc_h = hp  # heads 2hp (rows 0:64) and 2hp+1 (rows 64:128)
                esum = esump.tile([P, 2, TQ], BF16, tag="esum", name="esum")
                for sc in range(SC):
                    # both heads' scores in one 2-bank PSUM tile
                    ps = psum.tile([P, 2, 512], F32, tag="scores", name="ps_sc")
                    for j in range(2):
                        off = j * 64
                        nc.tensor.matmul(
                            ps[:, j],
                            k_sb[off:off + DH, kc_h, sc * P:(sc + 1) * P],
                            q_sb[off:off + DH, kc_h],
                            start=True, stop=True,
                        )
                    e_sb = epool.tile([P, 2, TQ], BF16, tag="e", name="e_sb")
                    nc.scalar.activation(e_sb, ps, AF.Exp)
                    nc.vector.tensor_mul(
                        e_sb, e_sb,
                        em_sb[:, sc:sc + 1, :].to_broadcast((P, 2, TQ)),
                    )
                    if sc == 0:
                        nc.vector.tensor_copy(esum, e_sb)
                    else:
                        nc.vector.tensor_add(esum, esum, e_sb)
                    for j in range(2):
                        h = 2 * hp + j
                        nc.tensor.matmul(
                            ps_o[j * 64:(j + 1) * 64],
                            v_sb[:, sc, h * DH:(h + 1) * DH], e_sb[:, j],
                            start=(sc == 0), stop=(sc == SC - 1),
                            tile_position=(0, j * 64),
                        )
                for j in range(2):
                    nc.tensor.matmul(
                        ps_z[j * 64:(j + 1) * 64], ones[:, :64], esum[:, j],
                        start=True, stop=True,
                        tile_position=(0, j * 64),
                    )
                rz = tmpp.tile([P, TQ], F32, tag="rz", name="rz")
                nc.vector.reciprocal_approx_fast(rz, ps_z)
                nc.vector.tensor_mul(attn_out[:, hp], ps_o, rz)
                if filler is not None:
                    filler(hp)

        def layer_norm(dst_fn, z_sb, li, post=None):
            """dst_fn(kc) <- (z - mu) * rstd [* g + b], feature-major.
            post(kc, ap) runs after each chunk is written."""
            ps_m = psum1.tile([P, 512], F32, tag="z", name="ps_m")
            for kc in range(KC):
                nc.tensor.matmul(ps_m, onesM, z_sb[:, kc],
                                 start=(kc == 0), stop=(kc == KC - 1))
            ps_s = psum1.tile([P, 512], F32, tag="o", name="ps_s")
            for kc in range(KC):
                zsq = lntp.tile([P, TQ], BF16, tag="zsq", name="zsq")
                nc.vector.tensor_mul(zsq, z_sb[:, kc], z_sb[:, kc])
                nc.tensor.matmul(ps_s, onesM, zsq,
                                 start=(kc == 0), stop=(kc == KC - 1))
            musq = tmpp.tile([P, TQ], F32, tag="stat", name="musq")
            nc.scalar.square(musq, ps_m)
            var = tmpp.tile([P, TQ], F32, tag="stat", name="var")
            nc.vector.tensor_sub(var, ps_s, musq)
            sd = tmpp.tile([P, TQ], F32, tag="stat", name="sd")
            nc.scalar.activation(sd, var, AF.Sqrt, bias=epsc)
            rstd = tmpp.tile([P, TQ], F32, tag="stat", name="rstd")
            nc.vector.reciprocal_approx_fast(rstd, sd)
            for kc in range(KC):
                t1 = lntp.tile([P, TQ], F32, tag="lnt", name="lnt")
                nc.vector.tensor_sub(t1, z_sb[:, kc], ps_m)
                d = dst_fn(kc)
                if flags["ln_affine"]:
                    t2 = lntp.tile([P, TQ], F32, tag="lnt2", name="lnt2")
                    nc.vector.tensor_mul(t2, t1, rstd)
                    nc.vector.tensor_scalar(
                        d, t2, ppc("g", li * KC + kc), ppc("b", li * KC + kc),
                        ALU.mult, ALU.add,
                    )
                else:
                    nc.vector.tensor_mul(d, t1, rstd)
                if post is not None:
                    post(kc, d)

        # ---------- self attention ----------
        x_sb = srcp.tile([P, KC, S], BF16, tag="srcx", name="x_sb")
        nc.sync.dma_start(x_sb, x_t)
        ms_sb = maskp.tile([P, SC, TQ], BF16, tag="mask", name="ms_sb")
        nc.sync.dma_start(ms_sb, m_self)
        em0_sb = ms_sb  # exp(mask) computed in place, chunk by chunk
        for sc in range(SC):
            nc.scalar.activation(em0_sb[:, sc], ms_sb[:, sc], AF.Exp)
        xq_view = x_sb[:, :, 0:TQ]  # own queries are the first TQ columns

        q_sb = kvp.tile([P, KC, TQ], BF16, tag="Q", name="q0")
        proj_fm(q_sb, wq[0], xq_view, KC, KC, TQ,
                bias=(lambda c: ppc("bq", c)) if flags["bq"] else None)
        k_sb = kvp.tile([P, KC, S], BF16, tag="K", name="k0")
        proj_fm(k_sb, wk[0], x_sb, KC, KC, S,
                bias=(lambda c: ppc("bk", c)) if flags["bk"] else None)
        v_sb = kvp.tile([P, SC, D], BF16, tag="V", name="v0")
        proj_tm(v_sb, wv[0], x_sb, 0)

        # cross/graph K/V (own token half only) get projected into the
        # AllGather bounce buffers; the PE work is interleaved (via fillers)
        # with the ACT-bound self-attention head loop, and each AllGather is
        # issued as soon as its half is staged.
        enc_sb = srcp.tile([P, KC, TQ], BF16, tag="srce", name="src1")
        nc.sync.dma_start(enc_sb, enc_t)
        gra_sb = srcp.tile([P, KC, TQ], BF16, tag="srcg", name="src2")
        nc.sync.dma_start(gra_sb, gra_t)
        all_groups = stage_groups(1, enc_sb) + stage_groups(2, gra_sb)

        def filler0(hp):
            n = len(all_groups)
            for g in all_groups[hp * n // 8:(hp + 1) * n // 8]:
                g()

        attn0 = attnp.tile([P, KC, TQ], BF16, tag="attn", name="attn0")
        attention(q_sb, k_sb, v_sb, em0_sb, attn0, filler=filler0)

        z1 = persist.tile([P, KC, TQ], BF16, tag="zres", name="z1")

        def evict_res0(ncn, ps, d):
            if flags["bo"]:
                t = tmpp.tile([P, TQ], F32, tag="evt", name="evt")
                nc.vector.tensor_scalar(t, ps, ppc("bo", ncn), None, ALU.add)
                nc.vector.tensor_add(d, t, x_sb[:, ncn, 0:TQ])
            else:
                nc.vector.tensor_add(d, ps, x_sb[:, ncn, 0:TQ])

        proj_fm(z1, wo[0], attn0, KC, KC, TQ, evict=evict_res0)
        h1 = persist.tile([P, KC, TQ], BF16, tag="h1")
        layer_norm(lambda kc: h1[:, kc], z1, 0)

        # ---------- cross + graph attention ----------
        h2 = persist.tile([P, KC, TQ], BF16, tag="h2")
        for a, m_d in ((1, m_enc), (2, m_gra)):
            m_sb = maskp.tile([P, SC, TQ], BF16, tag="mask", name=f"m{a}")
            nc.sync.dma_start(m_sb, m_d)
            em_sb = m_sb  # exp(mask) in place
            for sc in range(SC):
                nc.scalar.activation(em_sb[:, sc], m_sb[:, sc], AF.Exp)

            qa = kvp.tile([P, KC, TQ], BF16, tag="Q", name=f"q{a}")
            proj_fm(qa, wq[a], h1, KC, KC, TQ,
                    bias=(lambda c: ppc("bq", a * KC + c)) if flags["bq"] else None)
            ka = kvp.tile([P, KC, S], BF16, tag="K", name=f"k{a}")
            va = kvp.tile([P, SC, D], BF16, tag="V", name=f"v{a}")
            base_e = (a - 1) * 2 * D * TQ
            for sl in range(2):
                nc.sync.dma_start(
                    ka[:, :, sl * TQ:(sl + 1) * TQ],
                    kvout_all[sl, base_e:base_e + D * TQ].rearrange(
                        "(nc p t) -> p nc t", p=P, t=TQ))
                nc.sync.dma_start(
                    va[:, sl * (SC // 2):(sl + 1) * (SC // 2)],
                    kvout_all[sl, base_e + D * TQ:base_e + 2 * D * TQ].rearrange(
                        "(sc p n) -> p sc n", sc=SC // 2, p=P))

            attn_a = attnp.tile([P, KC, TQ], BF16, tag="attn", name=f"attn{a}")
            attention(qa, ka, va, em_sb, attn_a, filler=None)

            za = persist.tile([P, KC, TQ], BF16, tag="zres", name=f"za{a}")

            def evict_o(ncn, ps, d, a=a):
                if flags["bo"]:
                    nc.vector.tensor_scalar(d, ps, ppc("bo", a * KC + ncn),
                                            None, ALU.add)
                else:
                    nc.scalar.copy(d, ps)

            proj_fm(za, wo[a], attn_a, KC, KC, TQ, evict=evict_o)
            base = h1 if a == 1 else h2

            def post_add(kc, ap, base=base):
                nc.vector.tensor_add(h2[:, kc], base[:, kc], ap)

            layer_norm(
                lambda kc: lntp.tile([P, TQ], BF16, tag="lnc", name="lnc"),
                za, a, post=post_add,
            )

        # ---------- FFN ----------
        r_sb = persist.tile([P, FC, TQ], BF16, tag="r")
        proj_fm(r_sb, fc1, h2, FC, KC, TQ,
                bias=(lambda c: ppc("b1", c)) if flags["b1"] else None,
                relu=True)

        z3 = persist.tile([P, KC, TQ], BF16, tag="zres", name="z3")

        def evict_fc2(ncn, ps, d):
            if flags["b2"]:
                t = tmpp.tile([P, TQ], F32, tag="evt", name="evt2")
                nc.vector.tensor_scalar(t, ps, ppc("b2", ncn), None, ALU.add)
                nc.vector.tensor_add(d, t, h2[:, ncn])
            else:
                nc.vector.tensor_add(d, ps, h2[:, ncn])

        proj_fm(z3, fc2, r_sb, KC, FC, TQ, evict=evict_fc2, kq_split=4)

        out_r = out_t.rearrange("(kc p) t -> kc p t", p=P)
        layer_norm(
            lambda kc: lntp.tile([P, TQ], F32, tag="ochunk", name="ochunk"),
            z3, 3,
            post=lambda kc, ap: nc.sync.dma_start(out_r[kc], ap),
        )

    nc.finalize()
    return nc


def _pp_table(b_q, b_k, b_o, fc1_b, fc2_b, ln_g, ln_b):
    t = np.zeros((P, PP_COLS), np.float32)

    def put(name, vec):
        v = np.asarray(vec, np.float32).reshape(-1, P).T  # [128, n]
        t[:, PP_BASE[name]: PP_BASE[name] + v.shape[1]] = v

    put("bq", b_q.reshape(-1))
    put("bk", b_k.reshape(-1))
    put("bo", b_o.reshape(-1))
    put("b1", fc1_b)
    put("b2", fc2_b)
    put("g", ln_g.reshape(-1))
    put("b", ln_b.reshape(-1))
    return t


def _panels(w):
    """[Din, Dout] -> [Dout//128, 128(p), Din//128, 128(m)] partition-major
    column panels (each SBUF partition line is one contiguous run)."""
    din, dout = w.shape
    return np.ascontiguousarray(
        w.reshape(din // P, P, dout // P, P).transpose(2, 1, 0, 3)
    )


def _bf(a):
    return np.ascontiguousarray(np.asarray(a)).astype(ml_dtypes.bfloat16)


def _blob_put(blob, name, arr):
    o, n = _BLOB[name]
    a = np.ascontiguousarray(arr).reshape(-1)
    assert a.size == n, (name, a.size, n)
    blob[o:o + n] = a


def prepare(inputs):
    """Host-side prep: returns (flags, in_maps)."""
    ii = {k: np.asarray(v, np.float32) for k, v in inputs.items()}
    flags = {
        "bq": bool(np.any(ii["b_q"])),
        "bk": bool(np.any(ii["b_k"])),
        "bv": bool(np.any(ii["b_v"])),
        "bo": bool(np.any(ii["b_o"])),
        "b1": bool(np.any(ii["fc1_b"])),
        "b2": bool(np.any(ii["fc2_b"])),
        "ln_affine": bool(np.any(ii["ln_b"])
                          or not np.allclose(ii["ln_g"], 1.0)),
    }
    need_pp = any(flags.values())

    base = np.zeros(BLOB_ELEMS, ml_dtypes.bfloat16)
    _blob_put(base, "fc1", _bf(_panels(ii["fc1_w"])))
    _blob_put(base, "fc2", _bf(_panels(ii["fc2_w"])))
    for a in range(3):
        _blob_put(base, f"wq{a}", _bf(_panels(ii["W_q"][a] * SCALE)))
        _blob_put(base, f"wk{a}", _bf(_panels(ii["W_k"][a])))
        _blob_put(base, f"wv{a}", _bf(ii["W_v"][a]))
        _blob_put(base, f"wo{a}", _bf(_panels(ii["W_o"][a])))

    hid, enc, gra = (ii["hidden_states"], ii["enc_hidden_states"],
                     ii["graph_hidden_states"])
    msk = [ii["dec_self_mask"], ii["enc_dec_mask"], ii["graph_dec_mask"]]

    extra = {}
    if need_pp:
        extra["pp"] = _pp_table(ii["b_q"] * SCALE, ii["b_k"], ii["b_o"],
                                ii["fc1_b"], ii["fc2_b"], ii["ln_g"],
                                ii["ln_b"])
    if flags["bv"]:
        extra["bv"] = _bf(ii["b_v"])

    in_maps = []
    for c in range(NCORES):
        b, half = divmod(c, 2)
        r0 = half * TQ
        perm = np.r_[r0:S, 0:r0]  # own tokens first (self-attn key axis)
        blob = base.copy()
        _blob_put(blob, "x_t", _bf(hid[b].T[:, perm]))
        _blob_put(blob, "enc_t", _bf(enc[b].T[:, r0:r0 + TQ]))
        _blob_put(blob, "gra_t", _bf(gra[b].T[:, r0:r0 + TQ]))
        _blob_put(blob, "m_self", _bf(msk[0][b, 0].T[perm][:, r0:r0 + TQ]))
        _blob_put(blob, "m_enc", _bf(msk[1][b, 0].T[:, r0:r0 + TQ]))
        _blob_put(blob, "m_gra", _bf(msk[2][b, 0].T[:, r0:r0 + TQ]))
        in_maps.append({"blob": blob, **extra})
    return flags, in_maps


def get_program(flags):
    key = tuple(sorted(flags.items()))
    if key not in _cache:
        _cache[key] = build(flags)
    return _cache[key]


def gather(results):
    out = np.zeros((B, T, D), np.float32)
    for c in range(NCORES):
        b, half = divmod(c, 2)
        r0 = half * TQ
        out[b, r0:r0 + TQ, :] = results[c]["out_t"].T
    return out


def kernel(**inputs) -> np.ndarray:
    from concourse.bass_utils import run_bass_kernel_spmd

    flags, in_maps = prepare(inputs)
    nc = get_program(flags)
    res = run_bass_kernel_spmd(nc, in_maps, list(range(NCORES)))
    return gather(res.results)



# revision 12
# speedup vs baseline: 1.2559x; 1.1399x over previous
"""Trainium2 Bass kernel for nn_DecoderLayer_72327249264859.

Decoder layer: self-attn + (cross-attn || graph-attn) + FFN, each with
residual + layernorm. B=4, T=S=1024, D=1024, 16 heads, ffn=4096.

Sharding: pure data-parallel over query tokens. Core c handles batch
element b = c // 2, query rows (c % 2)*512 .. +512. Each core computes
full-length K/V for its batch element (2x duplicated K/V work, but zero
cross-core communication). The host permutes the self-attention key axis
so each core's own query tokens are the first 512 columns of x_t
(attention is invariant to key order as long as the mask rows are
permuted identically), so the same SPMD program works on every core.

Layout strategy: activations are kept *feature-major* in SBUF
([feature-dim on partitions, tokens on free]):
  - feature-major projections: lhsT = W column-panel [Din, 128] (natural),
    rhs = X_t [d, t] -> out [n, t]; no transposes anywhere;
  - token-major outputs (V for probs@V) use lhsT = X_t chunk (activations
    stationary) and rhs = W natural [Din, Dout];
  - scores are computed transposed ([s part, t free]): lhsT = K_t slice,
    rhs = Q_t slice (contraction over dh=64);
  - the additive mask (host-pre-transposed) is accumulated into the scores
    PSUM with an identity-matrix matmul;
  - softmax-Z (partition reduction) is an all-ones matmul that lands
    replicated across partitions in PSUM; 1/Z via fast DVE reciprocal;
  - layernorm mean / mean-of-squares via (1/D)-ones matmuls, same trick.
All matmuls bf16 with fp32 PSUM accumulation; heads processed in pairs
sharing PSUM tiles (partition halves).
"""

import sys

if "/opt/trn_rl_repo" not in sys.path:
    sys.path.insert(0, "/opt/trn_rl_repo")

import numpy as np
import ml_dtypes
from contextlib import ExitStack

import concourse.bacc as bacc
import concourse.mybir as mybir
from concourse.tile import TileContext

BF16 = mybir.dt.bfloat16
F32 = mybir.dt.float32
AF = mybir.ActivationFunctionType
ALU = mybir.AluOpType

B, T, S, D = 4, 1024, 1024, 1024
NH, DH = 16, 64
F = 4 * D
SCALE = DH**-0.5
EPS = 1e-5
P = 128
KC = D // P        # 8 feature chunks
SC = S // P        # 8 key chunks
FC = F // P        # 32 ffn chunks
TQ = 512           # query tokens per core
NCORES = 8

# per-partition parameter table layout (each param chunk = one column)
PP_BASE = {}
_off = 0
for _name, _n in (("bq", 24), ("bk", 24), ("bo", 24), ("b1", FC), ("b2", KC),
                  ("g", 32), ("b", 32)):
    PP_BASE[_name] = _off
    _off += _n
PP_COLS = _off

_cache = {}

# ---- packed input blob layout (element offsets, bf16) ----
# One flat bf16 tensor per core holds all weights + activations + masks so
# the SPMD executable has a single input parameter (per-exec dispatch cost
# scales with the argument count).
_BLOB = {}
_off_e = 0
def _blob_reg(name, n_elems):
    global _off_e
    _BLOB[name] = (_off_e, n_elems)
    _off_e += n_elems

for _a in range(3):
    _blob_reg(f"wq{_a}", D * D)
    _blob_reg(f"wk{_a}", D * D)
    _blob_reg(f"wv{_a}", D * D)
    _blob_reg(f"wo{_a}", D * D)
_blob_reg("fc1", D * F)
_blob_reg("fc2", F * D)
_blob_reg("x_t", D * S)
_blob_reg("enc_t", D * S)
_blob_reg("gra_t", D * S)
_blob_reg("m_self", S * TQ)
_blob_reg("m_enc", S * TQ)
_blob_reg("m_gra", S * TQ)
BLOB_ELEMS = _off_e


def build(flags):
    """Builds the per-core Bass program. flags control which (generically
    correct) bias/affine paths get emitted; for the reference inputs all
    biases are zero and gammas are one, so these stay off."""
    nc = bacc.Bacc()

    dp = nc.declare_dram_parameter
    blob = dp("blob", [BLOB_ELEMS], BF16, isOutput=False)

    def reg(name):
        o, n = _BLOB[name]
        return blob[o:o + n]

    def panels4(name, n_chunks, k_chunks):
        # weight panels, partition-major: [n_chunk, partition, k_chunk, m]
        return reg(name).rearrange("(n p k m) -> n p k m",
                                   n=n_chunks, p=P, k=k_chunks)

    x_t = reg("x_t").rearrange("(kc p t) -> p kc t", p=P, t=S)
    enc_t = reg("enc_t").rearrange("(kc p t) -> p kc t", p=P, t=S)
    gra_t = reg("gra_t").rearrange("(kc p t) -> p kc t", p=P, t=S)
    m_self = reg("m_self").rearrange("(sc p t) -> p sc t", p=P, t=TQ)
    m_enc = reg("m_enc").rearrange("(sc p t) -> p sc t", p=P, t=TQ)
    m_gra = reg("m_gra").rearrange("(sc p t) -> p sc t", p=P, t=TQ)
    wq = [panels4(f"wq{a}", KC, KC) for a in range(3)]
    wk = [panels4(f"wk{a}", KC, KC) for a in range(3)]
    wv = [reg(f"wv{a}").rearrange("(kc p n) -> p kc n", p=P, n=D)
          for a in range(3)]
    wo = [panels4(f"wo{a}", KC, KC) for a in range(3)]
    fc1 = panels4("fc1", FC, KC)
    fc2 = panels4("fc2", KC, FC)
    need_pp = any(flags.values())
    pp = dp("pp", [P, PP_COLS], F32, isOutput=False) if need_pp else None
    bv = dp("bv", [3, D], BF16, isOutput=False) if flags["bv"] else None
    out_t = dp("out_t", [D, TQ], F32, isOutput=True)
    # DRAM staging for cross/graph K/V (projected early, read back later)
    kst = [nc.dram_tensor(f"k_st{a}", [D, S], BF16) for a in (1, 2)]
    vst = [nc.dram_tensor(f"v_st{a}", [S, D], BF16) for a in (1, 2)]

    with TileContext(nc) as tc, ExitStack() as ctx:
        const = ctx.enter_context(tc.tile_pool(name="const", bufs=1))
        persist = ctx.enter_context(tc.tile_pool(name="persist", bufs=1))
        srcp = ctx.enter_context(tc.tile_pool(name="srcp", bufs=2))
        maskp = ctx.enter_context(tc.tile_pool(name="maskp", bufs=2))
        kvp = ctx.enter_context(tc.tile_pool(name="kvp", bufs=1))
        wpool = ctx.enter_context(tc.tile_pool(name="wpool", bufs=3))
        wvpool = ctx.enter_context(tc.tile_pool(name="wvpool", bufs=1))
        kvbp = ctx.enter_context(tc.tile_pool(name="kvbp", bufs=2))
        epool = ctx.enter_context(tc.tile_pool(name="epool", bufs=3))
        esump = ctx.enter_context(tc.tile_pool(name="esump", bufs=2))
        attnp = ctx.enter_context(tc.tile_pool(name="attnp", bufs=1))
        tmpp = ctx.enter_context(tc.tile_pool(name="tmpp", bufs=2))
        lntp = ctx.enter_context(tc.tile_pool(name="lntp", bufs=2))
        psum = ctx.enter_context(tc.tile_pool(name="psum", bufs=2, space="PSUM"))
        psum1 = ctx.enter_context(tc.tile_pool(name="psum1", bufs=1, space="PSUM"))

        ones = const.tile([P, P], BF16, tag="ones")
        nc.vector.memset(ones, 1.0)
        onesM = const.tile([P, P], BF16, tag="onesM")
        nc.vector.memset(onesM, 1.0 / D)
        epsc = const.tile([P, 1], F32, tag="epsc")
        nc.vector.memset(epsc, EPS)
        if need_pp:
            pp_sb = const.tile([P, PP_COLS], F32, tag="pp")
            nc.sync.dma_start(pp_sb, pp[:])
        if flags["bv"]:
            bv_sb = const.tile([3, D], BF16, tag="bv")
            nc.sync.dma_start(bv_sb, bv[:])

        def ppc(name, c):
            base = PP_BASE[name]
            return pp_sb[:, base + c: base + c + 1]

        # ---------- helpers ----------
        def proj_fm(dst, w_dram, rhs_sb, n_chunks, k_chunks, Tt,
                    bias=None, relu=False, evict=None, kq_split=1):
            """Feature-major projection: dst[:, ncn, tslice] = panel.T @ rhs.
            kq_split loads the k-dim weight panel in pieces (for big k)."""
            kq_n = k_chunks // kq_split
            assert kq_split == 1 or Tt == 512
            for ncn in range(n_chunks):
                shared_wp = None
                if kq_split == 1:
                    shared_wp = wpool.tile([P, k_chunks, P], BF16, tag="wp",
                                           name="wp")
                    nc.sync.dma_start(shared_wp, w_dram[ncn])
                for tn in range(Tt // 512):
                    ps = psum.tile([P, 512], F32, tag="proj", name="ps_proj")
                    for kq in range(kq_split):
                        if shared_wp is not None:
                            wp = shared_wp
                        else:
                            wp = wpool.tile([P, kq_n, P], BF16, tag="wp",
                                            name="wp")
                            nc.sync.dma_start(
                                wp, w_dram[ncn][:, kq * kq_n:(kq + 1) * kq_n]
                            )
                        for kl in range(kq_n):
                            kc = kq * kq_n + kl
                            nc.tensor.matmul(
                                ps, wp[:, kl],
                                rhs_sb[:, kc, tn * 512:(tn + 1) * 512],
                                start=(kc == 0), stop=(kc == k_chunks - 1),
                            )
                    d = dst[:, ncn, tn * 512:(tn + 1) * 512]
                    if evict is not None:
                        evict(ncn, ps, d)
                    elif relu:
                        if bias is not None:
                            nc.scalar.activation(d, ps, AF.Relu, bias=bias(ncn))
                        else:
                            nc.scalar.activation(d, ps, AF.Relu)
                    elif bias is not None:
                        nc.vector.tensor_scalar(d, ps, bias(ncn), None, ALU.add)
                    else:
                        nc.scalar.copy(d, ps)

        def proj_tm(v_sb, wv_dram, src_sb, a):
            """Token-major V projection: v_sb[:, sc, :]."""
            wv_sb = wvpool.tile([P, KC, D], BF16, tag="wv", name="wv_sb")
            nc.sync.dma_start(wv_sb, wv_dram)
            use_bias = flags["bv"]
            for sc in range(SC):
                for n2 in range(2):
                    ps = psum.tile([P, 512], F32, tag="proj", name="ps_v")
                    for kc in range(KC):
                        nc.tensor.matmul(
                            ps,
                            src_sb[:, kc, sc * P:(sc + 1) * P],
                            wv_sb[:, kc, n2 * 512:(n2 + 1) * 512],
                            start=(kc == 0),
                            stop=(kc == KC - 1 and not use_bias),
                        )
                    if use_bias:
                        nc.tensor.matmul(
                            ps, ones[0:1, :],
                            bv_sb[a:a + 1, n2 * 512:(n2 + 1) * 512],
                            start=False, stop=True,
                        )
                    nc.scalar.copy(v_sb[:, sc, n2 * 512:(n2 + 1) * 512], ps)

        def stage_groups(a, s_sb):
            """Emit-closures that project attention a's K/V into DRAM
            staging, group by group (each closure = one PSUM group)."""
            groups = []
            dst_k = kst[a - 1].rearrange("(nc p) t -> nc p t", p=P)
            use_kb = flags["bk"]

            # K: share one panel across the 2 tn groups
            panel_cache = {}

            def k_closure(ncn, tn):
                def run():
                    if ncn not in panel_cache:
                        wp = wpool.tile([P, KC, P], BF16, tag="wp", name="wp")
                        nc.sync.dma_start(wp, wk[a][ncn])
                        panel_cache[ncn] = wp
                    wp = panel_cache[ncn]
                    ps = psum.tile([P, 512], F32, tag="proj", name="ps_kst")
                    for kc in range(KC):
                        nc.tensor.matmul(
                            ps, wp[:, kc],
                            s_sb[:, kc, tn * 512:(tn + 1) * 512],
                            start=(kc == 0), stop=(kc == KC - 1),
                        )
                    bt = kvbp.tile([P, 512], BF16, tag="kvb", name="kvb")
                    if use_kb:
                        nc.vector.tensor_scalar(bt, ps, ppc("bk", a * KC + ncn),
                                                None, ALU.add)
                    else:
                        nc.scalar.copy(bt, ps)
                    nc.sync.dma_start(dst_k[ncn][:, tn * 512:(tn + 1) * 512], bt)

                return run

            for ncn in range(KC):
                for tn in range(2):
                    groups.append(k_closure(ncn, tn))

            dst_v = vst[a - 1].rearrange("(sc p) n -> sc p n", p=P)
            use_vb = flags["bv"]
            wv_holder = {}

            def v_closure(sc, n2):
                def run():
                    if "wv" not in wv_holder:
                        wv_sb = wvpool.tile([P, KC, D], BF16, tag="wv",
                                            name="wv_sb")
                        nc.sync.dma_start(wv_sb, wv[a])
                        wv_holder["wv"] = wv_sb
                    wv_sb = wv_holder["wv"]
                    ps = psum.tile([P, 512], F32, tag="proj", name="ps_vst")
                    for kc in range(KC):
                        nc.tensor.matmul(
                            ps,
                            s_sb[:, kc, sc * P:(sc + 1) * P],
                            wv_sb[:, kc, n2 * 512:(n2 + 1) * 512],
                            start=(kc == 0),
                            stop=(kc == KC - 1 and not use_vb),
                        )
                    if use_vb:
                        nc.tensor.matmul(
                            ps, ones[0:1, :],
                            bv_sb[a:a + 1, n2 * 512:(n2 + 1) * 512],
                            start=False, stop=True,
                        )
                    bt = kvbp.tile([P, 512], BF16, tag="kvb", name="kvb2")
                    nc.scalar.copy(bt, ps)
                    nc.sync.dma_start(dst_v[sc][:, n2 * 512:(n2 + 1) * 512], bt)

                return run

            for sc in range(SC):
                for n2 in range(2):
                    groups.append(v_closure(sc, n2))
            return groups

        def attention(q_sb, k_sb, v_sb, em_sb, attn_out, filler=None):
            """attn_out[:, hp, :] = softmax-attention, heads in pairs.
            em_sb holds exp(mask); the pair's two K=64 score matmuls are
            emitted back-to-back so they overlap in different PE row groups."""
            for hp in range(NH // 2):
                ps_z = psum1.tile([P, 512], F32, tag="z", name="ps_z")
                ps_o = psum1.tile([P, 512], F32, tag="o", name="ps_o")
                kc_h = hp  # heads 2hp (rows 0:64) and 2hp+1 (rows 64:128)
                esum = esump.tile([P, 2, TQ], BF16, tag="esum", name="esum")
                for sc in range(SC):
                    # both heads' scores in one 2-bank PSUM tile
                    ps = psum.tile([P, 2, 512], F32, tag="scores", name="ps_sc")
                    for j in range(2):
                        off = j * 64
                        nc.tensor.matmul(
                            ps[:, j],
                            k_sb[off:off + DH, kc_h, sc * P:(sc + 1) * P],
                            q_sb[off:off + DH, kc_h],
                            start=True, stop=True,
                        )
                    e_sb = epool.tile([P, 2, TQ], BF16, tag="e", name="e_sb")
                    nc.scalar.activation(e_sb, ps, AF.Exp)
                    nc.vector.tensor_mul(
                        e_sb, e_sb,
                        em_sb[:, sc:sc + 1, :].to_broadcast((P, 2, TQ)),
                    )
                    if sc == 0:
                        nc.vector.tensor_copy(esum, e_sb)
                    else:
                        nc.vector.tensor_add(esum, esum, e_sb)
                    for j in range(2):
                        h = 2 * hp + j
                        nc.tensor.matmul(
                            ps_o[j * 64:(j + 1) * 64],
                            v_sb[:, sc, h * DH:(h + 1) * DH], e_sb[:, j],
                            start=(sc == 0), stop=(sc == SC - 1),
                            tile_position=(0, j * 64),
                        )
                for j in range(2):
                    nc.tensor.matmul(
                        ps_z[j * 64:(j + 1) * 64], ones[:, :64], esum[:, j],
                        start=True, stop=True,
                        tile_position=(0, j * 64),
                    )
                rz = tmpp.tile([P, TQ], F32, tag="rz", name="rz")
                nc.vector.reciprocal_approx_fast(rz, ps_z)
                nc.vector.tensor_mul(attn_out[:, hp], ps_o, rz)
                if filler is not None:
                    filler(hp)

        def layer_norm(dst_fn, z_sb, li, post=None):
            """dst_fn(kc) <- (z - mu) * rstd [* g + b], feature-major.
            post(kc, ap) runs after each chunk is written."""
            ps_m = psum1.tile([P, 512], F32, tag="z", name="ps_m")
            for kc in range(KC):
                nc.tensor.matmul(ps_m, onesM, z_sb[:, kc],
                                 start=(kc == 0), stop=(kc == KC - 1))
            ps_s = psum1.tile([P, 512], F32, tag="o", name="ps_s")
            for kc in range(KC):
                zsq = lntp.tile([P, TQ], BF16, tag="zsq", name="zsq")
                nc.vector.tensor_mul(zsq, z_sb[:, kc], z_sb[:, kc])
                nc.tensor.matmul(ps_s, onesM, zsq,
                                 start=(kc == 0), stop=(kc == KC - 1))
            musq = tmpp.tile([P, TQ], F32, tag="stat", name="musq")
            nc.scalar.square(musq, ps_m)
            var = tmpp.tile([P, TQ], F32, tag="stat", name="var")
            nc.vector.tensor_sub(var, ps_s, musq)
            sd = tmpp.tile([P, TQ], F32, tag="stat", name="sd")
            nc.scalar.activation(sd, var, AF.Sqrt, bias=epsc)
            rstd = tmpp.tile([P, TQ], F32, tag="stat", name="rstd")
            nc.vector.reciprocal_approx_fast(rstd, sd)
            for kc in range(KC):
                t1 = lntp.tile([P, TQ], F32, tag="lnt", name="lnt")
                nc.vector.tensor_sub(t1, z_sb[:, kc], ps_m)
                d = dst_fn(kc)
                if flags["ln_affine"]:
                    t2 = lntp.tile([P, TQ], F32, tag="lnt2", name="lnt2")
                    nc.vector.tensor_mul(t2, t1, rstd)
                    nc.vector.tensor_scalar(
                        d, t2, ppc("g", li * KC + kc), ppc("b", li * KC + kc),
                        ALU.mult, ALU.add,
                    )
                else:
                    nc.vector.tensor_mul(d, t1, rstd)
                if post is not None:
                    post(kc, d)

        # ---------- self attention ----------
        x_sb = srcp.tile([P, KC, S], BF16, tag="src", name="x_sb")
        nc.sync.dma_start(x_sb, x_t)
        ms_sb = maskp.tile([P, SC, TQ], BF16, tag="mask", name="ms_sb")
        nc.sync.dma_start(ms_sb, m_self)
        em0_sb = ms_sb  # exp(mask) computed in place, chunk by chunk
        for sc in range(SC):
            nc.scalar.activation(em0_sb[:, sc], ms_sb[:, sc], AF.Exp)
        xq_view = x_sb[:, :, 0:TQ]  # own queries are the first TQ columns

        q_sb = kvp.tile([P, KC, TQ], BF16, tag="Q", name="q0")
        proj_fm(q_sb, wq[0], xq_view, KC, KC, TQ,
                bias=(lambda c: ppc("bq", c)) if flags["bq"] else None)
        k_sb = kvp.tile([P, KC, S], BF16, tag="K", name="k0")
        proj_fm(k_sb, wk[0], x_sb, KC, KC, S,
                bias=(lambda c: ppc("bk", c)) if flags["bk"] else None)
        v_sb = kvp.tile([P, SC, D], BF16, tag="V", name="v0")
        proj_tm(v_sb, wv[0], x_sb, 0)

        # cross/graph K/V get projected into DRAM staging; the PE work is
        # interleaved (via fillers) with the ACT-bound attention head loops.
        enc_sb = srcp.tile([P, KC, S], BF16, tag="src", name="src1")
        nc.sync.dma_start(enc_sb, enc_t)
        enc_groups = stage_groups(1, enc_sb)

        def filler0(hp):
            n = len(enc_groups)
            for g in enc_groups[hp * n // 8:(hp + 1) * n // 8]:
                g()

        attn0 = attnp.tile([P, KC, TQ], BF16, tag="attn", name="attn0")
        attention(q_sb, k_sb, v_sb, em0_sb, attn0, filler=filler0)

        z1 = persist.tile([P, KC, TQ], BF16, tag="zres", name="z1")

        def evict_res0(ncn, ps, d):
            if flags["bo"]:
                t = tmpp.tile([P, TQ], F32, tag="evt", name="evt")
                nc.vector.tensor_scalar(t, ps, ppc("bo", ncn), None, ALU.add)
                nc.vector.tensor_add(d, t, x_sb[:, ncn, 0:TQ])
            else:
                nc.vector.tensor_add(d, ps, x_sb[:, ncn, 0:TQ])

        proj_fm(z1, wo[0], attn0, KC, KC, TQ, evict=evict_res0)
        h1 = persist.tile([P, KC, TQ], BF16, tag="h1")
        layer_norm(lambda kc: h1[:, kc], z1, 0)

        # ---------- cross + graph attention ----------
        gra_sb = srcp.tile([P, KC, S], BF16, tag="src", name="src2")
        nc.sync.dma_start(gra_sb, gra_t)
        gra_groups = stage_groups(2, gra_sb)

        h2 = persist.tile([P, KC, TQ], BF16, tag="h2")
        for a, m_d in ((1, m_enc), (2, m_gra)):
            m_sb = maskp.tile([P, SC, TQ], BF16, tag="mask", name=f"m{a}")
            nc.sync.dma_start(m_sb, m_d)
            em_sb = m_sb  # exp(mask) in place
            for sc in range(SC):
                nc.scalar.activation(em_sb[:, sc], m_sb[:, sc], AF.Exp)

            qa = kvp.tile([P, KC, TQ], BF16, tag="Q", name=f"q{a}")
            proj_fm(qa, wq[a], h1, KC, KC, TQ,
                    bias=(lambda c: ppc("bq", a * KC + c)) if flags["bq"] else None)
            ka = kvp.tile([P, KC, S], BF16, tag="K", name=f"k{a}")
            nc.sync.dma_start(ka, kst[a - 1].rearrange("(kc p) t -> p kc t", p=P))
            va = kvp.tile([P, SC, D], BF16, tag="V", name=f"v{a}")
            nc.sync.dma_start(va, vst[a - 1].rearrange("(sc p) n -> p sc n", p=P))

            if a == 1:
                def filler1(hp):
                    n = len(gra_groups)
                    for g in gra_groups[hp * n // 8:(hp + 1) * n // 8]:
                        g()
            else:
                filler1 = None
            attn_a = attnp.tile([P, KC, TQ], BF16, tag="attn", name=f"attn{a}")
            attention(qa, ka, va, em_sb, attn_a, filler=filler1)

            za = persist.tile([P, KC, TQ], BF16, tag="zres", name=f"za{a}")

            def evict_o(ncn, ps, d, a=a):
                if flags["bo"]:
                    nc.vector.tensor_scalar(d, ps, ppc("bo", a * KC + ncn),
                                            None, ALU.add)
                else:
                    nc.scalar.copy(d, ps)

            proj_fm(za, wo[a], attn_a, KC, KC, TQ, evict=evict_o)
            base = h1 if a == 1 else h2

            def post_add(kc, ap, base=base):
                nc.vector.tensor_add(h2[:, kc], base[:, kc], ap)

            layer_norm(
                lambda kc: lntp.tile([P, TQ], BF16, tag="lnc", name="lnc"),
                za, a, post=post_add,
            )

        # ---------- FFN ----------
        r_sb = persist.tile([P, FC, TQ], BF16, tag="r")
        proj_fm(r_sb, fc1, h2, FC, KC, TQ,
                bias=(lambda c: ppc("b1", c)) if flags["b1"] else None,
                relu=True)

        z3 = persist.tile([P, KC, TQ], BF16, tag="zres", name="z3")

        def evict_fc2(ncn, ps, d):
            if flags["b2"]:
                t = tmpp.tile([P, TQ], F32, tag="evt", name="evt2")
                nc.vector.tensor_scalar(t, ps, ppc("b2", ncn), None, ALU.add)
                nc.vector.tensor_add(d, t, h2[:, ncn])
            else:
                nc.vector.tensor_add(d, ps, h2[:, ncn])

        proj_fm(z3, fc2, r_sb, KC, FC, TQ, evict=evict_fc2, kq_split=4)

        out_r = out_t.rearrange("(kc p) t -> kc p t", p=P)
        layer_norm(
            lambda kc: lntp.tile([P, TQ], F32, tag="ochunk", name="ochunk"),
            z3, 3,
            post=lambda kc, ap: nc.sync.dma_start(out_r[kc], ap),
        )

    nc.finalize()
    return nc


def _pp_table(b_q, b_k, b_o, fc1_b, fc2_b, ln_g, ln_b):
    t = np.zeros((P, PP_COLS), np.float32)

    def put(name, vec):
        v = np.asarray(vec, np.float32).reshape(-1, P).T  # [128, n]
        t[:, PP_BASE[name]: PP_BASE[name] + v.shape[1]] = v

    put("bq", b_q.reshape(-1))
    put("bk", b_k.reshape(-1))
    put("bo", b_o.reshape(-1))
    put("b1", fc1_b)
    put("b2", fc2_b)
    put("g", ln_g.reshape(-1))
    put("b", ln_b.reshape(-1))
    return t


def _panels(w):
    """[Din, Dout] -> [Dout//128, 128(p), Din//128, 128(m)] partition-major
    column panels (each SBUF partition line is one contiguous run)."""
    din, dout = w.shape
    return np.ascontiguousarray(
        w.reshape(din // P, P, dout // P, P).transpose(2, 1, 0, 3)
    )


def _bf(a):
    return np.ascontiguousarray(np.asarray(a)).astype(ml_dtypes.bfloat16)


def _blob_put(blob, name, arr):
    o, n = _BLOB[name]
    a = np.ascontiguousarray(arr).reshape(-1)
    assert a.size == n, (name, a.size, n)
    blob[o:o + n] = a


def prepare(inputs):
    """Host-side prep: returns (flags, in_maps)."""
    ii = {k: np.asarray(v, np.float32) for k, v in inputs.items()}
    flags = {
        "bq": bool(np.any(ii["b_q"])),
        "bk": bool(np.any(ii["b_k"])),
        "bv": bool(np.any(ii["b_v"])),
        "bo": bool(np.any(ii["b_o"])),
        "b1": bool(np.any(ii["fc1_b"])),
        "b2": bool(np.any(ii["fc2_b"])),
        "ln_affine": bool(np.any(ii["ln_b"])
                          or not np.allclose(ii["ln_g"], 1.0)),
    }
    need_pp = any(flags.values())

    base = np.zeros(BLOB_ELEMS, ml_dtypes.bfloat16)
    _blob_put(base, "fc1", _bf(_panels(ii["fc1_w"])))
    _blob_put(base, "fc2", _bf(_panels(ii["fc2_w"])))
    for a in range(3):
        _blob_put(base, f"wq{a}", _bf(_panels(ii["W_q"][a] * SCALE)))
        _blob_put(base, f"wk{a}", _bf(_panels(ii["W_k"][a])))
        _blob_put(base, f"wv{a}", _bf(ii["W_v"][a]))
        _blob_put(base, f"wo{a}", _bf(_panels(ii["W_o"][a])))

    hid, enc, gra = (ii["hidden_states"], ii["enc_hidden_states"],
                     ii["graph_hidden_states"])
    msk = [ii["dec_self_mask"], ii["enc_dec_mask"], ii["graph_dec_mask"]]

    extra = {}
    if need_pp:
        extra["pp"] = _pp_table(ii["b_q"] * SCALE, ii["b_k"], ii["b_o"],
                                ii["fc1_b"], ii["fc2_b"], ii["ln_g"],
                                ii["ln_b"])
    if flags["bv"]:
        extra["bv"] = _bf(ii["b_v"])

    in_maps = []
    for c in range(NCORES):
        b, half = divmod(c, 2)
        r0 = half * TQ
        perm = np.r_[r0:S, 0:r0]  # own tokens first (self-attn key axis)
        blob = base.copy()
        _blob_put(blob, "x_t", _bf(hid[b].T[:, perm]))
        _blob_put(blob, "enc_t", _bf(enc[b].T))
        _blob_put(blob, "gra_t", _bf(gra[b].T))
        _blob_put(blob, "m_self", _bf(msk[0][b, 0].T[perm][:, r0:r0 + TQ]))
        _blob_put(blob, "m_enc", _bf(msk[1][b, 0].T[:, r0:r0 + TQ]))
        _blob_put(blob, "m_gra", _bf(msk[2][b, 0].T[:, r0:r0 + TQ]))
        in_maps.append({"blob": blob, **extra})
    return flags, in_maps


def get_program(flags):
    key = tuple(sorted(flags.items()))
    if key not in _cache:
        _cache[key] = build(flags)
    return _cache[key]


def gather(results):
    out = np.zeros((B, T, D), np.float32)
    for c in range(NCORES):
        b, half = divmod(c, 2)
        r0 = half * TQ
        out[b, r0:r0 + TQ, :] = results[c]["out_t"].T
    return out


def kernel(**inputs) -> np.ndarray:
    from concourse.bass_utils import run_bass_kernel_spmd

    flags, in_maps = prepare(inputs)
    nc = get_program(flags)
    res = run_bass_kernel_spmd(nc, in_maps, list(range(NCORES)))
    return gather(res.results)

